# revision 2
# baseline (speedup 1.0000x reference)
"""Trainium2 Bass kernel for nn_CAGKE_learnable_minmax (v4).

Same math as v3 (scale-invariant minmax drops softmax-Z and 1/sqrt(2pi) from
the conv path; weff = e^w/|s|; one Exp for the taps; conv as two PSUM-
accumulated matmuls against a 256-wide Hankel of the collapsed kernel geff).

Hankel build: geff row (512B) SBUF->DRAM into a host-supplied zero-padded
scratch row (ExternalInput, so the zero flanks are pre-resident), then ONE
DRAM->SBUF overlapping-window read [[1,128],[1,256]] -> rtB[k,u] = g[k+u].
(SBUF->SBUF overlap APs are silently broken on hardware: the DGE iterator
keeps per-step byte offsets only mod 16B.)

Inputs: cxs [128,8] chain scalars (lands first, starts the sigma chain ~1us
earlier), cxb [128,320] consts+masks, gscr [1,640] zeros, noise [1,8192].
"""

import os

import numpy as np

import concourse.bass as bass
import concourse.bacc as bacc
import concourse.mybir as mybir
import concourse.tile as tile
from concourse.bass_utils import run_bass_kernel_spmd

T = 8192
D = 128
NB = T // 128
INV_SQRT_2PI = 0.39894228
NOISE_SIGMA = 0.01
F32 = mybir.dt.float32
BF16 = mybir.dt.bfloat16
F32R = mybir.dt.float32r
AX = mybir.AxisListType
ALU = mybir.AluOpType
ACT = mybir.ActivationFunctionType

# cxs column layout (chain scalars, [128, 8])
S_DF = 0     # d/127
S_ONE = 1    # ones column (zpp rhs)
S_WT = 2     # w^T
S_SMIN = 3   # smin replicated
S_SMAX = 4   # smax replicated
S_W = 8

# cxb column layout ([128, 320])
C_U2 = 0     # [0:128] (x-64)^2 grid
C_XA = 128   # [128:192] mask window A
C_XB = 192   # [192:256] mask window B
C_OR = 256   # [256:320] ones ROW (row 0 only; z64 lhsT)
C_W = 320

GS = 640     # gscr scratch row length (zeros; geff written at [128:256])


def _emit(tc, nc, h, mm="f32r", swdge=False):
    sb_cm = tc.tile_pool(name="sb", bufs=1)
    pp_cm = tc.tile_pool(name="ps", bufs=1, space="PSUM")
    sb = sb_cm.__enter__()
    pp = pp_cm.__enter__()

    if mm == "bf16":
        MST = BF16
    elif mm == "f32r":
        MST = F32R
    else:
        MST = F32

    cxs = sb.tile([128, S_W], F32, tag="cxs")
    cxb = sb.tile([128, C_W], F32, tag="cxb")
    nz = sb.tile([NB, 128], F32, tag="nz")
    expw = sb.tile([128, 1], F32, tag="expw")
    stp = sb.tile([128, 1], F32, tag="stp")
    sg = sb.tile([128, 1], F32, tag="sg")
    rsg = sb.tile([128, 1], F32, tag="rsg")
    amp = sb.tile([128, 1], F32, tag="amp")
    nh2 = sb.tile([128, 1], F32, tag="nh2")
    weff = sb.tile([128, 1], MST, tag="weff")
    expt = sb.tile([128, 128], MST, tag="expt")
    g1 = sb.tile([1, 128], MST, tag="g1")
    rtB = sb.tile([128, 256], MST, tag="rtB")
    mA = sb.tile([128, 64], MST, tag="mA")
    mB = sb.tile([128, 64], MST, tag="mB")
    zsb = sb.tile([1, 1], F32, tag="zsb")
    nz01 = sb.tile([NB, 128], F32, tag="nz01")
    ps = sb.tile([NB, 128], F32, tag="ps")
    mmx = sb.tile([NB, 2], F32, tag="mmx")
    pr = sb.tile([NB, 2], F32, tag="pr")
    rng = sb.tile([NB, 1], F32, tag="rng")
    inv = sb.tile([NB, 1], F32, tag="inv")
    outx = sb.tile([NB, 128], F32, tag="outx")

    gp = pp.tile([1, 128], F32, tag="gp")
    zpp = pp.tile([1, 1], F32, tag="zpp")
    z64 = pp.tile([NB, 1], F32, tag="z64")
    cp = pp.tile([NB, 128], F32, tag="cp")

    dF127 = cxs[:, S_DF:S_DF + 1]
    onescol = cxs[:, S_ONE:S_ONE + 1]
    wT = cxs[:, S_WT:S_WT + 1]
    sminR = cxs[:, S_SMIN:S_SMIN + 1]
    smaxR = cxs[:, S_SMAX:S_SMAX + 1]
    u2 = cxb[:, C_U2:C_U2 + 128]
    xa = cxb[:, C_XA:C_XA + 64]
    xb = cxb[:, C_XB:C_XB + 64]
    onesrow = cxb[0:1, C_OR:C_OR + 64]

    # ---- input DMAs ------------------------------------------------------
    nc.sync.dma_start(out=cxs, in_=bass.AP(h["cxs"], 0, [[S_W, 128], [1, S_W]]))
    nc.scalar.dma_start(out=cxb, in_=bass.AP(h["cxb"], 0, [[C_W, 128], [1, C_W]]))
    nc.scalar.dma_start(out=nz, in_=bass.AP(h["noise"], 0, [[128, NB], [1, 128]]))

    # ---- sigma / weight chain -------------------------------------------
    nc.scalar.activation(out=expw, in_=wT, func=ACT.Exp)
    nc.vector.tensor_sub(out=stp, in0=smaxR, in1=sminR)
    nc.vector.tensor_scalar(
        out=sg, in0=dF127, scalar1=stp, scalar2=sminR, op0=ALU.mult, op1=ALU.add,
    )
    nc.vector.reciprocal(out=rsg, in_=sg)
    nc.vector.tensor_scalar(
        out=nh2, in0=rsg, scalar1=rsg, scalar2=-0.5, op0=ALU.mult, op1=ALU.mult,
    )
    nc.scalar.activation(out=amp, in_=rsg, func=ACT.Abs)
    nc.vector.tensor_mul(out=weff, in0=expw, in1=amp)
    nc.scalar.activation(out=expt, in_=u2, func=ACT.Exp, scale=nh2)

    # ---- masks ----------------------------------------------------------
    nc.vector.tensor_scalar(out=mA, in0=xa, scalar1=0.5, scalar2=None, op0=ALU.is_gt)
    nc.vector.tensor_scalar(out=mB, in0=xb, scalar1=0.5, scalar2=None, op0=ALU.is_gt)

    # ---- geff row; DRAM roundtrip Hankel --------------------------------
    nc.tensor.matmul(gp, lhsT=weff, rhs=expt, start=True, stop=True)
    nc.vector.tensor_copy(out=g1, in_=gp)
    wr = nc.gpsimd if swdge else nc.scalar
    rd = nc.gpsimd if swdge else nc.sync
    wr.dma_start(out=bass.AP(h["gscr"], 128, [[1, 128]]), in_=g1)
    rd.dma_start(out=rtB, in_=bass.AP(h["gscr"], 0, [[1, 128], [1, 256]]))

    # ---- Z and noise rescale (off critical path) ------------------------
    nc.tensor.matmul(zpp, lhsT=expw, rhs=onescol, start=True, stop=True)
    nc.vector.tensor_copy(out=zsb, in_=zpp)
    nc.tensor.matmul(z64, lhsT=onesrow, rhs=zsb, start=True, stop=True)
    nc.vector.tensor_scalar(
        out=nz01, in0=nz, scalar1=z64, scalar2=NOISE_SIGMA / INV_SQRT_2PI,
        op0=ALU.mult, op1=ALU.mult,
    )

    # ---- conv -----------------------------------------------------------
    nc.tensor.matmul(cp, lhsT=mA, rhs=rtB[:, 128:256], start=True, stop=False)
    nc.tensor.matmul(cp, lhsT=mB, rhs=rtB[:, 0:128], start=False, stop=True)

    # ---- + noise; minmax; normalize -------------------------------------
    nc.vector.tensor_add(out=ps, in0=cp, in1=nz01)
    nc.vector.reduce_max(out=mmx[:, 0:1], in_=ps, axis=AX.X)
    nc.vector.tensor_reduce(out=mmx[:, 1:2], in_=ps, axis=AX.X, op=ALU.min, negate=True)
    from concourse import bass_isa
    nc.gpsimd.partition_all_reduce(pr, mmx, channels=NB, reduce_op=bass_isa.ReduceOp.max)
    nc.vector.tensor_add(out=rng, in0=pr[:, 0:1], in1=pr[:, 1:2])
    nc.vector.reciprocal(out=inv, in_=rng)
    nc.vector.tensor_scalar(
        out=outx, in0=ps, scalar1=pr[:, 1:2], scalar2=inv, op0=ALU.add, op1=ALU.mult,
    )
    nc.scalar.dma_start(out=bass.AP(h["out"], 0, [[128, NB], [1, 128]]), in_=outx)

    sb_cm.__exit__(None, None, None)
    pp_cm.__exit__(None, None, None)


def build_nc(mm="f32r", swdge=False):
    nc = bacc.Bacc("TRN2", debug=False, enable_partition_id=False)
    gdt = BF16 if mm == "bf16" else (F32R if mm == "f32r" else F32)
    h = {
        "cxs": nc.dram_tensor("cxs", [128, S_W], F32, kind="ExternalInput"),
        "cxb": nc.dram_tensor("cxb", [128, C_W], F32, kind="ExternalInput"),
        "gscr": nc.dram_tensor("gscr", [1, GS], gdt, kind="ExternalInput"),
        "noise": nc.dram_tensor("noise", [1, T], F32, kind="ExternalInput"),
        "out": nc.dram_tensor("out", [1, T], F32, kind="ExternalOutput"),
    }
    with tile.TileContext(nc) as tc:
        _emit(tc, nc, h, mm=mm, swdge=swdge)
    nc.compile()
    return nc


_NC_CACHE = None
_CONFIG = {"mm": "f32r", "swdge": False}


def _get_nc():
    global _NC_CACHE
    if _NC_CACHE is None:
        _NC_CACHE = build_nc(**_CONFIG)
    return _NC_CACHE


def _prep_inputs(inputs):
    """Layout-only host prep plus input-independent constants."""
    X = np.asarray(inputs["X"], dtype=np.float32)
    weight = np.asarray(inputs["weight"], dtype=np.float32)
    smin = np.asarray(inputs["sigma_min"], dtype=np.float32)
    smax = np.asarray(inputs["sigma_max"], dtype=np.float32)
    noise = np.asarray(inputs["noise"], dtype=np.float32)

    xf = X.reshape(T)
    xpa = np.concatenate([np.zeros(64, np.float32), xf])[:T]
    xpb = np.concatenate([xf[64:], np.zeros(64, np.float32)])
    xra = np.ascontiguousarray(xpa.reshape(NB, 128)[:, ::-1].T)  # m[128b+63-k]
    xrb = np.ascontiguousarray(xpb.reshape(NB, 128)[:, ::-1].T)  # m[128b+191-k]

    cxs = np.zeros((128, S_W), np.float32)
    cxs[:, S_DF] = np.arange(128, dtype=np.float32) / (D - 1)
    cxs[:, S_ONE] = 1.0
    cxs[:, S_WT] = weight.reshape(D)
    cxs[:, S_SMIN] = smin[0]
    cxs[:, S_SMAX] = smax[0]

    cxb = np.zeros((128, C_W), np.float32)
    cxb[:, C_U2:C_U2 + 128] = ((np.arange(128, dtype=np.float32) - 64.0) ** 2)[None, :]
    cxb[:, C_XA:C_XA + 64] = xra
    cxb[:, C_XB:C_XB + 64] = xrb
    cxb[0, C_OR:C_OR + 64] = 1.0

    mm = _CONFIG["mm"]
    gdt = np.dtype(np.float32)
    gscr = np.zeros((1, GS), np.uint16 if mm == "bf16" else np.float32)

    return {
        "cxs": cxs,
        "cxb": cxb,
        "gscr": gscr,
        "noise": np.ascontiguousarray(noise.reshape(1, T)),
    }


def kernel(**inputs: np.ndarray) -> np.ndarray:
    nc = _get_nc()
    in_map = _prep_inputs(inputs)
    n_cores = 8
    res = run_bass_kernel_spmd(nc, [in_map] * n_cores, core_ids=list(range(n_cores)))
    return res.results[0]["out"].reshape(1, T)
